# revision 1
# baseline (speedup 1.0000x reference)
"""Delta-form spectral kernel for nn_Dynamics_2748779069592 (TRN2, 8 cores).

Identity: Out_n = Z0 + Qc[(g16^n - 1) .* W0]Qc^T + F_n,
          F_n = Qc[(DT*sum_{k<16n} g^k) .* Qtil]Qc^T
|g16^n - 1| <= 0.0203, so tf32's 4.9e-4 relative rounding contributes ~1e-5
of |Z| to the output. F_n and Qtil (O(0.26) magnitude) stay fp32.

Sharding: 8 cores as 2x4 grid — core c owns 8 batch elems (half c%2) and 4
output times (quarter c//2). No cross-core communication.

Inputs are host-preswizzled to the [128, 512] on-chip layout (partition p
holds grid rows p and p+128) so every input DMA is 128 contiguous 2KB reads.
Engine map: PE matmuls; ACT all PSUM->SBUF copies; DVE elementwise muls +
PSUM-add evacuations; GPSIMD the (F_j + z_e) SBUF adds.
"""
import sys

sys.path.insert(0, "/opt/trn_rl_repo")
import warnings

warnings.filterwarnings("ignore")
import numpy as np

N = 256
P = 128
NE = 8  # elems per core
NT = 4  # output times per core
NCORES = 8
DT = 1e-3
NU = 1e-2

_compiled = None


def tf32_round(x):
    u = np.asarray(x, dtype=np.float32).view(np.uint32).astype(np.uint64)
    r = ((u >> 13) + ((u >> 12) & 1)) << 13
    return (r & 0xFFFFFFFF).astype(np.uint32).view(np.float32)


def swz(x):
    """[..., 256, 256] -> [..., 128, 512] on-chip layout (rows p, p+128)."""
    sh = x.shape[:-2]
    return (
        x.reshape(sh + (2, P, N)).swapaxes(-3, -2).reshape(sh + (P, 2 * N))
    )


def _make_tables():
    C = np.zeros((N, N))
    i = np.arange(N)
    C[i, (i + 1) % N] = 1.0
    C[i, (i - 1) % N] = 1.0
    C[i, i] = -2.0
    lam, Qc = np.linalg.eigh(C)
    a = DT * NU
    g = 1.0 + a * (lam[:, None] + lam[None, :])
    an = np.empty((16, N, N))
    bn = np.empty((16, N, N))
    S = np.zeros_like(g)
    gk = np.ones_like(g)
    for k in range(16 * 16):
        S += gk
        gk *= g
        if (k + 1) % 16 == 0:
            t = (k + 1) // 16 - 1
            an[t] = gk - 1.0
            bn[t] = DT * S
    g16 = g**16
    # per time-quarter tq (n0 = 4*tq): A = g16^n0 - 1, B = g16^n0 * (g16 - 1)
    at = np.empty((4, N, N))
    bt = np.empty((4, N, N))
    for tq in range(4):
        gn0 = g16 ** (4 * tq)
        at[tq] = gn0 - 1.0
        bt[tq] = gn0 * (g16 - 1.0)
    return Qc, an, bn, at, bt


def _build():
    import concourse.bacc as bacc
    import concourse.mybir as mybir
    from concourse.tile import TileContext

    f32 = mybir.dt.float32
    f32r = mybir.dt.float32r
    nc = bacc.Bacc("TRN2", target_bir_lowering=False, debug=False)

    z_d = nc.dram_tensor("z", [NE, P, 2 * N], f32, kind="ExternalInput")
    zr_d = nc.dram_tensor("zr", [NE, P, 2 * N], f32r, kind="ExternalInput")
    q_d = nc.dram_tensor("q", [P, 2 * N], f32, kind="ExternalInput")
    qc_d = nc.dram_tensor("qc", [P, 2 * N], f32, kind="ExternalInput")
    qct_d = nc.dram_tensor("qct", [P, 2 * N], f32, kind="ExternalInput")
    qcr_d = nc.dram_tensor("qcr", [P, 2 * N], f32r, kind="ExternalInput")
    qctr_d = nc.dram_tensor("qctr", [P, 2 * N], f32r, kind="ExternalInput")
    at_d = nc.dram_tensor("at", [P, 2 * N], f32r, kind="ExternalInput")
    bt_d = nc.dram_tensor("bt", [P, 2 * N], f32r, kind="ExternalInput")
    bn_d = nc.dram_tensor("bn", [NT, P, 2 * N], f32, kind="ExternalInput")
    out_d = nc.dram_tensor("out", [NE, NT, P, 2 * N], f32, kind="ExternalOutput")

    with TileContext(nc) as tc:
        with (
            tc.tile_pool(name="const", bufs=1) as cpool,
            tc.tile_pool(name="zs", bufs=1) as zpool,
            tc.tile_pool(name="work", bufs=2) as wpool,
            tc.tile_pool(name="recycle", bufs=5) as rpool,
            tc.tile_pool(name="i1p", bufs=8) as i1pool,
            tc.tile_pool(name="ddp", bufs=6) as ddpool,
            tc.tile_pool(name="hp", bufs=9) as hpool,
            tc.tile_pool(name="vp", bufs=1) as vpool,
            tc.tile_pool(name="gp", bufs=12) as gpool,
            tc.tile_pool(name="jvp", bufs=8) as jvpool,
            tc.tile_pool(name="outp", bufs=8) as opool,
            tc.tile_pool(name="psum", bufs=8, space="PSUM") as psum,
        ):
            _uid = [0]

            def nm(tag):
                _uid[0] += 1
                return f"{tag}_{_uid[0]}"

            def loadc(pool, tag, dt_, dram_ap):
                t = pool.tile([P, 2 * N], dt_, tag=tag, name=nm(tag))
                nc.sync.dma_start(out=t[:, :], in_=dram_ap)
                return t

            # order matters for the head: Qtil chain needs q+qc, forwards need zr+qcr
            q_t = loadc(cpool, "q", f32, q_d.ap()[:, :])
            qc_t = loadc(cpool, "qc", f32, qc_d.ap()[:, :])
            qcr_t = loadc(cpool, "qcr", f32r, qcr_d.ap()[:, :])
            qctr_t = loadc(cpool, "qctr", f32r, qctr_d.ap()[:, :])
            qct_t = loadc(cpool, "qct", f32, qct_d.ap()[:, :])
            zr_t = [loadc(rpool, "zr", f32r, zr_d.ap()[e]) for e in range(NE)]
            at_t = loadc(cpool, "at", f32r, at_d.ap()[:, :])
            bt_t = loadc(cpool, "bt", f32r, bt_d.ap()[:, :])
            bn_t = [loadc(cpool, f"bn{j}", f32, bn_d.ap()[j]) for j in range(NT)]
            z_t = [loadc(rpool, "z", f32, z_d.ap()[e]) for e in range(NE)]

            def mm256(lhs_t, rhs_t, out_t, evac):
                """out = lhs.T @ rhs (256x256 mats in [128, 512] layout)."""
                for m in range(2):
                    pt = psum.tile([P, N], f32, tag="ps", name=nm("ps"))
                    for k in range(2):
                        nc.tensor.matmul(
                            pt[:, :],
                            lhs_t[:, N * k + P * m : N * k + P * m + P],
                            rhs_t[:, N * k : N * k + N],
                            start=(k == 0),
                            stop=(k == 1),
                        )
                    evac(out_t[:, N * m : N * m + N], pt[:, :])
                return out_t

            act_cp = lambda o, p: nc.scalar.copy(out=o, in_=p)
            dve_cp = lambda o, p: nc.vector.tensor_copy(o, p)

            # ---- f32r forward transforms: W0_e ----
            w0_t = []
            for e in range(NE):
                fm = mm256(zr_t[e], qcr_t, i1pool.tile([P, 2 * N], f32r, tag="i1", name=nm("fm")), act_cp)
                w0 = mm256(fm, qcr_t, rpool.tile([P, 2 * N], f32r, tag="w0", name=nm("w0")), act_cp)
                w0_t.append(w0)

            # ---- per elem e: U = Inv(A.*W0), V = Inv(B.*W0); h = z + U;
            #      then 4 outputs: Out_je = (j+1)*V + (F_j + h) ----
            jv_all = {}
            h_all = {}

            def uv_chain(e):
                dv = ddpool.tile([P, 2 * N], f32r, tag="dv", name=nm("dv"))
                dveng = nc.gpsimd if e >= 3 else nc.vector
                dveng.tensor_mul(dv[:, :], bt_t[:, :], w0_t[e][:, :])
                du = ddpool.tile([P, 2 * N], f32r, tag="du", name=nm("du"))
                nc.gpsimd.tensor_mul(du[:, :], at_t[:, :], w0_t[e][:, :])
                iu = mm256(du, qctr_t, i1pool.tile([P, 2 * N], f32r, tag="i1", name=nm("iu")), act_cp)
                # U stage-2: fold h = z + U into the PSUM evacuation (DVE)
                h_e = hpool.tile([P, 2 * N], f32, tag="h", name=nm("h"))
                for m in range(2):
                    pt = psum.tile([P, N], f32, tag="ps", name=nm("ps"))
                    for k in range(2):
                        nc.tensor.matmul(
                            pt[:, :],
                            iu[:, N * k + P * m : N * k + P * m + P],
                            qctr_t[:, N * k : N * k + N],
                            start=(k == 0),
                            stop=(k == 1),
                        )
                    nc.vector.tensor_add(
                        h_e[:, N * m : N * m + N], pt[:, :], z_t[e][:, N * m : N * m + N]
                    )
                iv = mm256(dv, qctr_t, i1pool.tile([P, 2 * N], f32r, tag="i1", name=nm("iv")), act_cp)
                v_e = vpool.tile([P, 2 * N], f32, tag=f"v{e}", name=nm("v"))
                mm256(iv, qctr_t, v_e, act_cp)
                jv_all[e] = v_e
                h_all[e] = h_e



            # ---- fp32 Qtil first (short PE prefix), F inverses after uv(0) ----
            m1q = mm256(q_t, qc_t, wpool.tile([P, 2 * N], f32, tag="m1q", name=nm("m1q")), act_cp)
            qtil = mm256(m1q, qc_t, wpool.tile([P, 2 * N], f32, tag="qtil", name=nm("qtil")), act_cp)

            for e in range(1):
                uv_chain(e)

            f_t = []
            for j in range(NT):
                rb = wpool.tile([P, 2 * N], f32, tag="rb", name=nm("rb"))
                nc.gpsimd.tensor_mul(rb[:, :], bn_t[j][:, :], qtil[:, :])
                f1 = mm256(rb, qct_t, wpool.tile([P, 2 * N], f32, tag="f1", name=nm("f1")), act_cp)
                f_t.append(mm256(f1, qct_t, cpool.tile([P, 2 * N], f32, tag=f"F{j}", name=nm("F")), act_cp))

            for e in range(1, NE):
                uv_chain(e)

            # ---- assembly phase (low priority; fills engine idle) ----
            for e in range(NE):
                v_e, h_e = jv_all[e], h_all[e]
                for j in range(NT):
                    g_t = gpool.tile([P, 2 * N], f32, tag="g", name=nm("g"))
                    nc.vector.tensor_add(g_t[:, :], f_t[j][:, :], h_e[:, :])
                    if j == 0:
                        src = v_e
                    else:
                        src = jvpool.tile([P, 2 * N], f32, tag="jv", name=nm("jv"))
                        nc.scalar.mul(src[:, :], v_e[:, :], float(j + 1))
                    o_t = opool.tile([P, 2 * N], f32, tag="o", name=nm("o"))
                    nc.vector.tensor_add(o_t[:, :], src[:, :], g_t[:, :])
                    nc.sync.dma_start(
                        out=out_d.ap()[e, j],
                        in_=o_t[:, :],
                    )

    nc.compile()
    return nc


def _get_compiled():
    global _compiled
    if _compiled is None:
        _compiled = _build()
    return _compiled


def _run(inputs_full, Q, trace=False):
    from concourse import bass_utils

    nc = _get_compiled()
    Qc, an, bn, at, bt = _make_tables()
    qc32 = Qc.astype(np.float32)
    qct32 = np.ascontiguousarray(Qc.T).astype(np.float32)
    z32 = np.ascontiguousarray(inputs_full.astype(np.float32))
    zs = swz(z32)
    zrs = tf32_round(zs)
    qs, qcs, qcts = swz(np.asarray(Q, np.float32)), swz(qc32), swz(qct32)
    bns = swz(bn)
    ats, bts = swz(at), swz(bt)
    in_maps = []
    for c in range(NCORES):
        eh = c % 2
        tq = c // 2
        in_maps.append(
            {
                "z": np.ascontiguousarray(zs[eh * NE : (eh + 1) * NE]),
                "zr": np.ascontiguousarray(zrs[eh * NE : (eh + 1) * NE]),
                "q": np.ascontiguousarray(qs),
                "qc": np.ascontiguousarray(qcs),
                "qct": np.ascontiguousarray(qcts),
                "qcr": tf32_round(qcs),
                "qctr": tf32_round(qcts),
                "at": tf32_round(np.ascontiguousarray(ats[tq])),
                "bt": tf32_round(np.ascontiguousarray(bts[tq])),
                "bn": np.ascontiguousarray(bns[tq * NT : (tq + 1) * NT]).astype(np.float32),
            }
        )
    kw = dict(trace=True) if trace else {}
    last_err = None
    for attempt in range(3):
        try:
            res = bass_utils.run_bass_kernel_spmd(
                nc, in_maps, core_ids=list(range(NCORES)), **kw
            )
            break
        except Exception as exc:  # rare transient device error; retry
            last_err = exc
            import time

            time.sleep(5)
    else:
        raise last_err
    out = np.empty((16, 16, N, N), dtype=np.float32)
    for c in range(NCORES):
        eh, tq = c % 2, c // 2
        r = res.results[c]["out"]  # [NE, NT, 128, 512] swizzled
        r = r.reshape(NE, NT, P, 2, N).swapaxes(2, 3).reshape(NE, NT, N, N)
        out[eh * NE : (eh + 1) * NE, tq * NT : (tq + 1) * NT] = r
    return out, res


def kernel(inputs, Q):
    inputs = np.ascontiguousarray(np.asarray(inputs, dtype=np.float32))
    Q = np.ascontiguousarray(np.asarray(Q, dtype=np.float32))
    out, _ = _run(inputs, Q, trace=False)
    return out



# revision 6
# speedup vs baseline: 2.4183x; 2.4183x over previous
"""Linearized spectral kernel for nn_Dynamics_2748779069592 (TRN2, 8 cores).

Out_n = Z0 + [(g16^n - 1).*W0] back-transformed + F_n. Over n=1..16,
g16^n - 1 = n*(g16-1) + O(2e-4), and F_n = 0.016n*Q + O(7e-3 absolute,
below the bf16 rounding floor of the output). So per batch elem:

    Out_{e,n} = z_e + n*Vp_e,   Vp_e = H[(g16-1).*(H z_e H)]H + 0.016*Q

H is the Hartley matrix (symmetric orthogonal, diagonalizes periodic
circulants). All compute in bf16 (rel err ~5.7e-3 vs the 2e-2 gate);
host converts dtypes and layouts.

Sharding: pure data parallel, core c owns batch elems {2c, 2c+1}, all 16
output times. Elems processed sequentially so elem-0's output assembly
(DVE scalar_tensor_tensor) overlaps elem-1's PE transform chain.
Outputs are written as [128,1024] pairs of consecutive time steps so
each DMA moves 2KB/partition rows across the 16 DMA engines.
"""
import sys

sys.path.insert(0, "/opt/trn_rl_repo")
import warnings

warnings.filterwarnings("ignore")
import numpy as np

N = 256
P = 128
NE = 2  # elems per core
NT = 16  # output times per core
NCORES = 8
DT = 1e-3
NU = 1e-2

_compiled = None


def swz(x):
    """[..., 256, 256] -> [..., 128, 512] tile layout (partition p holds
    grid rows p and p+128 in free-dim halves)."""
    sh = x.shape[:-2]
    return x.reshape(sh + (2, P, N)).swapaxes(-3, -2).reshape(sh + (P, 2 * N))


def _make_tables():
    import ml_dtypes

    j = np.arange(N)
    H = (np.cos(2 * np.pi * np.outer(j, j) / N) + np.sin(2 * np.pi * np.outer(j, j) / N)) / np.sqrt(N)
    lam = -2.0 + 2.0 * np.cos(2 * np.pi * j / N)
    a = DT * NU
    g = 1.0 + a * (lam[:, None] + lam[None, :])
    d16 = g**16 - 1.0
    return swz(H).astype(ml_dtypes.bfloat16), swz(d16).astype(ml_dtypes.bfloat16)


def _build():
    import concourse.bacc as bacc
    import concourse.mybir as mybir
    from concourse.tile import TileContext

    bf16 = mybir.dt.bfloat16
    f32 = mybir.dt.float32
    mult = mybir.AluOpType.mult
    add = mybir.AluOpType.add
    nc = bacc.Bacc("TRN2", target_bir_lowering=False, debug=False)

    # consts packed: [qc | qs | d16], each [128, 512]
    cst_d = nc.dram_tensor("cst", [P, 3 * 512], bf16, kind="ExternalInput")
    zp_d = nc.dram_tensor("zp", [P, 1024], bf16, kind="ExternalInput")
    # out[e, pair, :, j*512+c] = output for elem e at time n = 2*pair+j+1
    out_d = nc.dram_tensor("out", [NE, NT // 2, P, 1024], bf16, kind="ExternalOutput")

    with TileContext(nc) as tc:
        with (
            tc.tile_pool(name="const", bufs=1) as cpool,
            tc.tile_pool(name="work", bufs=8) as wpool,
            tc.tile_pool(name="vp", bufs=2) as vpool,
            tc.tile_pool(name="op", bufs=6) as opool,
            tc.tile_pool(name="psum", bufs=8, space="PSUM") as psum,
        ):
            _uid = [0]

            def nm(tag):
                _uid[0] += 1
                return f"{tag}_{_uid[0]}"

            cst = cpool.tile([P, 3 * 512], bf16, tag="cst", name="cst")
            nc.scalar.dma_start(out=cst[:, :], in_=cst_d.ap()[:, :])
            zp = cpool.tile([P, 1024], bf16, tag="zp", name="zp")
            nc.sync.dma_start(out=zp[:, :], in_=zp_d.ap()[:, :])
            qc = cst[:, 0:512]
            qs = cst[:, 512:1024]
            d16 = cst[:, 1024:1536]

            def mm256(lhs, rhs):
                """psum[128,512] = lhs_mat^T @ rhs_mat (tile-form operands)."""
                pt = psum.tile([P, 512], f32, tag="ps", name=nm("ps"))
                for m in range(2):
                    for k in range(2):
                        nc.tensor.matmul(
                            pt[:, 256 * m : 256 * m + 256],
                            lhs[:, 256 * k + 128 * m : 256 * k + 128 * m + 128],
                            rhs[:, 256 * k : 256 * k + 256],
                            start=(k == 0),
                            stop=(k == 1),
                        )
                return pt

            def wtile(tag):
                return wpool.tile([P, 512], bf16, tag=tag, name=nm(tag))

            for e in range(NE):
                z_e = zp[:, 512 * e : 512 * e + 512]
                ps = mm256(z_e, qc)  # fm = z^T H
                fm = wtile("fm")
                nc.scalar.copy(out=fm[:, :], in_=ps[:, :])
                ps = mm256(fm, qc)  # W0 = H z H; fold d16 at evac
                d_e = wtile("d")
                nc.vector.tensor_mul(d_e[:, :], d16[:, :], ps[:, :])
                ps = mm256(d_e, qc)  # iv = D^T H
                iv = wtile("iv")
                nc.scalar.copy(out=iv[:, :], in_=ps[:, :])
                ps = mm256(iv, qc)  # V = H D H; fold +Qs at evac
                vp = vpool.tile([P, 512], bf16, tag="vp", name=nm("vp"))
                nc.vector.tensor_add(vp[:, :], ps[:, :], qs)

                # ---- 16 outputs for this elem: O_n = n*Vp + z ----
                for pair in range(NT // 2):
                    o_t = opool.tile([P, 1024], bf16, tag="o", name=nm("o"))
                    for j in range(2):
                        n = 2 * pair + j + 1
                        nc.vector.scalar_tensor_tensor(
                            o_t[:, 512 * j : 512 * j + 512],
                            vp[:, :],
                            float(n),
                            z_e,
                            op0=mult,
                            op1=add,
                        )
                    dma_eng = nc.sync if pair % 2 == 0 else nc.scalar
                    dma_eng.dma_start(out=out_d.ap()[e, pair], in_=o_t[:, :])

    nc.compile()
    return nc


def _get_compiled():
    global _compiled
    if _compiled is None:
        _compiled = _build()
    return _compiled


def _run(inputs_full, Q, trace=False):
    import ml_dtypes
    from concourse import bass_utils

    bf = ml_dtypes.bfloat16
    nc = _get_compiled()
    qc_t, d16_t = _make_tables()
    qs_t = swz(0.016 * np.asarray(Q, np.float64)).astype(bf)
    cst = np.concatenate([qc_t, qs_t, d16_t], axis=1)
    zs = swz(np.asarray(inputs_full, np.float32)).astype(bf)  # [16,128,512]
    in_maps = []
    for c in range(NCORES):
        zpair = np.ascontiguousarray(
            np.concatenate([zs[2 * c], zs[2 * c + 1]], axis=1)
        )
        in_maps.append({"cst": cst, "zp": zpair})
    kw = dict(trace=True) if trace else {}
    last_err = None
    for attempt in range(3):
        try:
            res = bass_utils.run_bass_kernel_spmd(
                nc, in_maps, core_ids=list(range(NCORES)), **kw
            )
            break
        except Exception as exc:  # rare transient device error; retry
            last_err = exc
            import time

            time.sleep(5)
    else:
        raise last_err
    # per core: [NE, 8, 128, 1024] bf16; free idx = j*512 + h*256 + col
    R = np.stack([np.asarray(res.results[c]["out"]) for c in range(NCORES)])
    R = R.astype(np.float32)  # [8, 2, 8, 128, 1024]
    R = R.reshape(NCORES, NE, NT // 2, P, 2, 2, N)  # [c, e, pair, p, j, h, col]
    R = R.transpose(0, 1, 2, 4, 5, 3, 6)  # [c, e, pair, j, h, p, col]
    out = np.ascontiguousarray(R.reshape(16, NT, 2 * P, N))
    return out, res


def kernel(inputs, Q):
    inputs = np.ascontiguousarray(np.asarray(inputs, dtype=np.float32))
    Q = np.ascontiguousarray(np.asarray(Q, dtype=np.float32))
    out, _ = _run(inputs, Q, trace=False)
    return out
